# revision 2
# baseline (speedup 1.0000x reference)
"""Trainium2 Bass kernel for EquivariantGraphConv message passing.

Math (reference):
    scalar = x[:,0,:]; vector = x[:,1:,:].reshape(N, 3H)
    scalar_out = scalar @ Wsr.T + b + segsum(scalar[col] @ Wsrel.T, row)
    vector_out = vector @ Wvr.T + segsum(vector[col] @ Wvrel.T, row)

Key identity used: the edge transform is linear, so
    segsum(feat[col] @ W.T, row) == segsum(feat[col], row) @ W.T
We therefore aggregate the raw 512-dim node features per destination first
(16x fewer matmul FLOPs), then apply all four weight matrices per *node*.

Sharding: destinations are sharded across the 8 cores (1280 nodes each, in
10 chunks of 128).  Edges are sorted by destination on the host, so each
core only consumes its own edge shard and no cross-core reduction is
needed.  Each core gathers source features from a replicated padded node
table in DRAM with one big indirect DMA per chunk, builds one-hot
"selection" matrices on the vector engine (row_in_chunk == iota) and
matmul-accumulates P^T @ G into PSUM to realize the segment sum.

Precision/speed trick: the gathered node row is stored as a 1KB fp8 pair
[hi(512) | lo(512)] with hi = e4m3(x), lo = e4m3(x - hi).  Stage-1 runs as
fp8 DoubleRow matmuls (2 k-tiles per instruction, 2x PE throughput vs
bf16) and hi+lo recovers better-than-bf16 accuracy while keeping the
gather at the same 1KB/edge as a bf16 row.
"""

import os
import sys

sys.path.insert(0, "/opt/trn_rl_repo")

import numpy as np
import ml_dtypes

import concourse.bass as bass
import concourse.mybir as mybir
import concourse.tile as tile
from concourse.bacc import Bacc
from concourse.bass_utils import run_bass_kernel_spmd

N_NODES = 10000
N_EDGES = 160000
H = 128
F = 4 * H            # 512 features per node (scalar 128 + vector 384)
GB = 2 * F           # 1024 fp8 bytes per gathered row: [hi(512) | lo(512)]
P = 128              # partitions
NP_PAD = 10240       # padded node count (80 chunks of 128)
N_CORES = 8
NODES_PER_CORE = NP_PAD // N_CORES       # 1280
CHUNKS_PER_CORE = NODES_PER_CORE // P    # 10
N_CHUNKS = NP_PAD // P                   # 80
ZERO_ROW = N_NODES                       # padded zero row used by dummy edges
DEFAULT_T = 17                           # edge tiles per chunk (17*128 = 2176 cap)
GBUFS = 4                                # G tile pool depth

CFG = "fp8hilo"

# test.py hooks
PROFILE = {"on": False, "trace_cores": None, "last": None}

_prog_cache = {}

FP8 = mybir.dt.float8e4
BF16 = mybir.dt.bfloat16
NP_FP8 = ml_dtypes.float8_e4m3
NP_BF16 = ml_dtypes.bfloat16


def _build_program(T):
    """Build the (SPMD, per-core-identical) Bass program."""
    T_pad = T + (T & 1)       # stage-1 consumes k-tile pairs (DoubleRow)
    ND = T_pad // 2

    nc = Bacc("TRN2", num_swdge_queues=4)
    f32 = mybir.dt.float32

    xg = nc.dram_tensor("xg", [NP_PAD, GB], FP8, kind="ExternalInput")
    # dma_gather is limited to ~1024 descriptors per instruction; split each
    # chunk's T*128 indices into NQ pieces of <= GQ indices.
    GQ = 1024
    NQ = (T * P + GQ - 1) // GQ
    WQ = GQ // 16  # idx columns per piece in the 16-partition wrapped layout
    cols = nc.dram_tensor("cols", [P, CHUNKS_PER_CORE * NQ * WQ],
                          mybir.dt.int16, kind="ExternalInput")
    rr = nc.dram_tensor("rr", [P, CHUNKS_PER_CORE * T_pad], BF16,
                        kind="ExternalInput")
    xt = nc.dram_tensor("xt", [P, 4 * NODES_PER_CORE], BF16,
                        kind="ExternalInput")
    wsrel = nc.dram_tensor("wsrel", [P, H], BF16, kind="ExternalInput")
    wsroot = nc.dram_tensor("wsroot", [P, H], BF16, kind="ExternalInput")
    wvrel = nc.dram_tensor("wvrel", [P, 3 * 384], BF16, kind="ExternalInput")
    wvroot = nc.dram_tensor("wvroot", [P, 3 * 384], BF16, kind="ExternalInput")
    bias = nc.dram_tensor("bias", [P, H], f32, kind="ExternalInput")
    iota = nc.dram_tensor("iota", [P, P], BF16, kind="ExternalInput")
    ident = nc.dram_tensor("ident", [P, P], BF16, kind="ExternalInput")
    out = nc.dram_tensor("out", [NODES_PER_CORE, F], f32, kind="ExternalOutput")

    DR = mybir.MatmulPerfMode.DoubleRow

    with tile.TileContext(nc) as tc:
        with (
            tc.tile_pool(name="consts", bufs=1) as cpool,
            tc.tile_pool(name="edges", bufs=6) as epool,
            tc.tile_pool(name="gbuf", bufs=GBUFS) as gpool,
            tc.tile_pool(name="work", bufs=4) as wpool,
            tc.tile_pool(name="pagg", bufs=3, space="PSUM") as pagg,
            tc.tile_pool(name="pmisc", bufs=2, space="PSUM") as pmisc,
        ):
            # constants go on the Scalar HWDGE queue so the first chunk's
            # index DMAs (sync queue) aren't stuck behind the 2.6MB xt load
            xt_sb = cpool.tile([P, 4 * NODES_PER_CORE], BF16)
            nc.scalar.dma_start(xt_sb[:], xt[:])
            wsrel_sb = cpool.tile([P, H], BF16)
            nc.scalar.dma_start(wsrel_sb[:], wsrel[:])
            wsroot_sb = cpool.tile([P, H], BF16)
            nc.scalar.dma_start(wsroot_sb[:], wsroot[:])
            wvrel_sb = cpool.tile([P, 3 * 384], BF16)
            nc.scalar.dma_start(wvrel_sb[:], wvrel[:])
            wvroot_sb = cpool.tile([P, 3 * 384], BF16)
            nc.scalar.dma_start(wvroot_sb[:], wvroot[:])
            bias_sb = cpool.tile([P, H], f32)
            nc.scalar.dma_start(bias_sb[:], bias[:])
            iota_sb = cpool.tile([P, P], BF16)
            nc.scalar.dma_start(iota_sb[:], iota[:])
            ident_sb = cpool.tile([P, P], BF16)
            nc.scalar.dma_start(ident_sb[:], ident[:])

            LAG = 2  # stage-2 for chunk c-LAG runs amid stage-1 of chunk c
            agg_tiles = {}

            # all chunks' indices in two up-front DMAs (first thing on sync)
            cols_all = cpool.tile([P, CHUNKS_PER_CORE * NQ * WQ], mybir.dt.int16)
            nc.sync.dma_start(cols_all[:], cols[:])
            rr_all = cpool.tile([P, CHUNKS_PER_CORE * T_pad], BF16)
            nc.sync.dma_start(rr_all[:], rr[:])

            def stage1(c):
                cols_sb = cols_all[:, c * NQ * WQ:(c + 1) * NQ * WQ]
                rr_sb = rr_all[:, c * T_pad:(c + 1) * T_pad]

                # gather: edge i -> G[i % 128, i // 128, :] = xg[cols_flat[i], :]
                G = gpool.tile([P, T_pad * GB], FP8, tag="G")
                for q in range(NQ):
                    nidx = min(GQ, T * P - q * GQ)
                    nslots = nidx // P
                    nc.gpsimd.dma_gather(
                        G[:, q * (GQ // P) * GB:
                             (q * (GQ // P) + nslots) * GB]
                        .rearrange("p (t f) -> p t f", f=GB),
                        xg[:],
                        cols_sb[:, q * WQ:(q + 1) * WQ],
                        nidx,
                        nidx,
                        GB,
                        queue_num=(c * NQ + q) % 4,
                    )
                if T_pad > T and c < GBUFS:
                    # pad tile: zero once per physical pool buffer; gathers
                    # never write it, so it stays zero on later rotations
                    nc.scalar.memzero(G[:, T * GB:T_pad * GB])

                # one-hot P[p, t*128 + d] = (rr[p, t] == d); pad tiles have
                # rr = -1 so their columns are all-zero
                Pm = epool.tile([P, T_pad * P], FP8, tag="P")
                for t in range(T_pad):
                    nc.vector.tensor_tensor(
                        out=Pm[:, t * P:(t + 1) * P],
                        in0=rr_sb[:, t:t + 1].to_broadcast([P, P]),
                        in1=iota_sb[:],
                        op=mybir.AluOpType.is_equal,
                    )

                # segment-sum: agg[d, f] = sum_t P_t^T @ G_t as fp8 DoubleRow
                # (2 k-tiles per matmul); hi and lo halves share the one-hot
                agg_ps = pagg.tile([P, F], f32, tag="agg")
                P3 = Pm.rearrange("p (t d) -> p t d", d=P)
                G3 = G.rearrange("p (t f) -> p t f", f=GB)
                for d in range(ND):
                    lhsT = P3[:, 2 * d:2 * d + 2, :]
                    nc.tensor.matmul(
                        out=agg_ps[:],
                        lhsT=lhsT,
                        rhs=G3[:, 2 * d:2 * d + 2, 0:F],
                        start=(d == 0),
                        stop=False,
                        perf_mode=DR,
                    )
                    nc.tensor.matmul(
                        out=agg_ps[:],
                        lhsT=lhsT,
                        rhs=G3[:, 2 * d:2 * d + 2, F:GB],
                        start=False,
                        stop=(d == ND - 1),
                        perf_mode=DR,
                    )
                agg_sb = wpool.tile([P, F], BF16, tag="aggsb")
                nc.scalar.copy(agg_sb[:], agg_ps[:])
                agg_tiles[c] = agg_sb

            def stage2(c):
                agg_sb = agg_tiles.pop(c)
                # transpose agg -> aggT[f, d] (4 PE transposes of 128x128)
                aggT_ps = pmisc.tile([P, F], BF16, tag="aggT")
                for fc in range(4):
                    nc.tensor.transpose(
                        out=aggT_ps[:, fc * P:(fc + 1) * P],
                        in_=agg_sb[:, fc * P:(fc + 1) * P],
                        identity=ident_sb[:],
                    )
                aggT_sb = wpool.tile([P, F], BF16, tag="aggTsb")
                nc.scalar.copy(aggT_sb[:], aggT_ps[:])

                # stage 2: out[d, :128]  = agg_s @ WsrelT + x_s @ WsrootT (+bias)
                #          out[d, 128:]  = agg_v @ WvrelT + x_v @ WvrootT
                osv_ps = pmisc.tile([P, F], f32, tag="osv")
                nc.tensor.matmul(out=osv_ps[:, 0:H],
                                 lhsT=aggT_sb[:, 0:P], rhs=wsrel_sb[:],
                                 start=True, stop=False)
                nc.tensor.matmul(out=osv_ps[:, 0:H],
                                 lhsT=xt_sb[:, c * P:(c + 1) * P],
                                 rhs=wsroot_sb[:],
                                 start=False, stop=True)
                for kc in range(3):
                    nc.tensor.matmul(
                        out=osv_ps[:, H:F],
                        lhsT=aggT_sb[:, (1 + kc) * P:(2 + kc) * P],
                        rhs=wvrel_sb[:, kc * 384:(kc + 1) * 384],
                        start=(kc == 0), stop=False)
                for kc in range(3):
                    nc.tensor.matmul(
                        out=osv_ps[:, H:F],
                        lhsT=xt_sb[:, (1 + kc) * NODES_PER_CORE + c * P:
                                      (1 + kc) * NODES_PER_CORE + (c + 1) * P],
                        rhs=wvroot_sb[:, kc * 384:(kc + 1) * 384],
                        start=False, stop=(kc == 2))

                out_sb = wpool.tile([P, F], f32, tag="outsb")
                nc.vector.tensor_add(out_sb[:, 0:H], osv_ps[:, 0:H], bias_sb[:])
                nc.scalar.copy(out_sb[:, H:F], osv_ps[:, H:F])
                nc.sync.dma_start(out[c * P:(c + 1) * P, :], out_sb[:])

            for c in range(CHUNKS_PER_CORE + LAG):
                if c < CHUNKS_PER_CORE:
                    stage1(c)
                if c >= LAG:
                    stage2(c - LAG)

    nc.finalize()
    return nc


def _get_program(T):
    if T not in _prog_cache:
        _prog_cache[T] = _build_program(T)
    return _prog_cache[T]


def kernel(x, edge_index, W_scalar_rel, W_scalar_root, b_scalar_root,
           W_vector_rel, W_vector_root):
    x = np.asarray(x, dtype=np.float32)
    n = x.shape[0]
    assert n == N_NODES, x.shape
    row = np.asarray(edge_index[0], dtype=np.int64)
    col = np.asarray(edge_index[1], dtype=np.int64)

    # ---- host-side shard construction (sort edges by destination) ----
    order = np.argsort(row, kind="stable")
    row_s = row[order]
    col_s = col[order]
    bounds = np.searchsorted(row_s, np.arange(0, NP_PAD + 1, P))
    counts = np.diff(bounds)
    T = max(DEFAULT_T, int(np.ceil(counts.max() / P)))
    T_pad = T + (T & 1)

    cap = T * P
    # padding edges point at the all-zero ZERO_ROW and rr=-1 (never matches
    # the iota, so their one-hot column is all-zero)
    cols_pad = np.full((N_CHUNKS, cap), ZERO_ROW, dtype=np.int16)
    rr_pad = np.full((N_CHUNKS, T_pad * P), -1.0, dtype=np.float32)
    for g in range(N_CHUNKS):
        s, e = bounds[g], bounds[g + 1]
        m = e - s
        if m:
            cols_pad[g, :m] = col_s[s:e]
            rr_pad[g, :m] = (row_s[s:e] - g * P).astype(np.float32)
    # dma_gather: flat edge i -> partition i % 128, tile-slot i // 128.
    # Each chunk's indices are split into NQ pieces of <= 1024; within a
    # piece, idx element j lives at wrapped position [j % 16, j // 16],
    # and the 16-row block is replicated across all 128 partitions
    # (the tx/rx Q7 cores each read their own 16-partition copy).
    GQ = 1024
    NQ = (cap + GQ - 1) // GQ
    WQ = GQ // 16
    cols_q = np.zeros((N_CHUNKS, NQ, GQ), dtype=np.int16)
    cols_q.reshape(N_CHUNKS, NQ * GQ)[:, :cap] = cols_pad
    wrap = cols_q.reshape(N_CHUNKS, NQ, WQ, 16).transpose(0, 1, 3, 2)  # [.., 16, WQ]
    cols_wrapped = np.tile(wrap, (1, 1, 8, 1))  # [N_CHUNKS, NQ, 128, WQ]
    cols_flat = cols_wrapped.transpose(0, 2, 1, 3).reshape(N_CHUNKS, P, NQ * WQ)
    # per core: [P, CHUNKS*NQ*WQ] with chunk-major free dim
    cols_arr = np.ascontiguousarray(
        cols_flat.reshape(N_CORES, CHUNKS_PER_CORE, P, NQ * WQ)
        .transpose(0, 2, 1, 3).reshape(N_CORES, P, CHUNKS_PER_CORE * NQ * WQ))
    # rr for edge i goes to [i % 128, i // 128]; per core [P, CHUNKS*T_pad]
    rr_arr = rr_pad.reshape(N_CHUNKS, T_pad, P).transpose(0, 2, 1)
    rr_arr = np.ascontiguousarray(
        rr_arr.reshape(N_CORES, CHUNKS_PER_CORE, P, T_pad)
        .transpose(0, 2, 1, 3).reshape(N_CORES, P, CHUNKS_PER_CORE * T_pad))
    rr_arr = rr_arr.astype(NP_BF16)

    x_flat = np.zeros((NP_PAD, F), dtype=np.float32)
    x_flat[:n] = x.reshape(n, F)
    x_hi = x_flat.astype(NP_FP8)
    x_lo = (x_flat - x_hi.astype(np.float32)).astype(NP_FP8)
    xg_full = np.ascontiguousarray(
        np.concatenate([x_hi, x_lo], axis=1))  # [NP_PAD, 1024] fp8

    xT = x_flat.T  # [512, 10240] for the root transform

    wsrelT = np.ascontiguousarray(np.asarray(W_scalar_rel, np.float32).T).astype(NP_BF16)
    wsrootT = np.ascontiguousarray(np.asarray(W_scalar_root, np.float32).T).astype(NP_BF16)
    wvrelT = np.ascontiguousarray(np.asarray(W_vector_rel, np.float32).T)
    wvrootT = np.ascontiguousarray(np.asarray(W_vector_root, np.float32).T)
    wvrel_packed = np.concatenate(
        [wvrelT[kc * P:(kc + 1) * P, :] for kc in range(3)], axis=1).astype(NP_BF16)
    wvroot_packed = np.concatenate(
        [wvrootT[kc * P:(kc + 1) * P, :] for kc in range(3)], axis=1).astype(NP_BF16)
    bias_t = np.ascontiguousarray(
        np.broadcast_to(np.asarray(b_scalar_root, np.float32), (P, H)))
    iota_t = np.ascontiguousarray(
        np.broadcast_to(np.arange(P, dtype=np.float32), (P, P))).astype(NP_BF16)
    ident_t = np.eye(P, dtype=np.float32).astype(NP_BF16)

    in_maps = []
    for core in range(N_CORES):
        base = core * NODES_PER_CORE
        xTc = xT[:, base:base + NODES_PER_CORE]  # [512, 1280]
        xTr = np.ascontiguousarray(
            xTc.reshape(4, P, NODES_PER_CORE).transpose(1, 0, 2)
               .reshape(P, 4 * NODES_PER_CORE)).astype(NP_BF16)
        in_maps.append({
            "xg": xg_full,
            "cols": np.ascontiguousarray(cols_arr[core]),
            "rr": np.ascontiguousarray(rr_arr[core]),
            "xt": xTr,
            "wsrel": wsrelT,
            "wsroot": wsrootT,
            "wvrel": wvrel_packed,
            "wvroot": wvroot_packed,
            "bias": bias_t,
            "iota": iota_t,
            "ident": ident_t,
        })

    nc = _get_program(T)
    kw = {}
    if PROFILE["on"]:
        kw = dict(trace=True, trace_cores=PROFILE["trace_cores"])
    res = run_bass_kernel_spmd(nc, in_maps, list(range(N_CORES)), **kw)
    PROFILE["last"] = res

    out_full = np.concatenate([res.results[i]["out"] for i in range(N_CORES)],
                              axis=0)
    return np.ascontiguousarray(
        out_full[:N_NODES].reshape(N_NODES, 4, H).astype(np.float32))
